# revision 41
# baseline (speedup 1.0000x reference)
"""DeepseekV2 MoE layer on 8 Trainium2 NeuronCores (Bass/Tile, SPMD).

Strategy (expert-parallel with split-expert load balancing, bf16 matmuls):
 - Host computes the MoE gate routing in numpy (matches the jax reference:
   top-k margins are ~1e-4, far above ulp noise).
 - Routed experts run in S=3 uniform "slots" per core (SPMD needs uniform
   shapes).  Slot capacities (C0 >= C1 >= C2) are chosen by a small exact
   DP so that the 24 cells (8 cores x 3 slots) can hold all 16 experts,
   splitting a large expert's tokens across several cells.  This cuts the
   per-core padded token count from ~1040 (2-slot scheme) to ~816.
 - All matmul operands are bf16 (full PE rate, half the HBM traffic of
   fp32; rel err ~5e-3 vs the 2e-2 gate).  PSUM accumulates fp32.
 - GEMM2 is computed transposed: stationary = w_down^T tiles, moving =
   activations [i, tokens].  Output lands as [D, cap] (host transposes),
   so compute scales exactly with cap and the combine-weight scale is a
   single fused psum->sbuf multiply against a host-broadcast [128, cap]
   weight tile.
 - Shared expert is TP-sharded over its intermediate dim (352 rows per
   core, padded to 384), same transposed-GEMM2 scheme, summed on host.

The schedule is DMA/compute balanced (per core ~72MB at ~300GB/s vs
~241us of matmul rows), so queue placement is what the tuning is about:
 - sync HW DGE queue: wgu stream + all 16 w_down tiles per slot,
   interleaved via a hook inside GEMM1 (no consumption gating).
 - scalar HW DGE queue: x-side prefetch (throttled one phase ahead by the
   ACT instructions that sit between the triggers) + batched outputs.
 - gpsimd SW DGE queue (1us/transfer overhead): only the single resident
   shared-w_down transfer.
 - Shared-expert GEMM1 runs FIRST (weight-light warm-up curtain: deep
   slot-0 prefetch + HAM clock ramp), shared GEMM2 LAST (weight-free
   phase under which the output queues drain, split across both HW
   queues).
Measured: 283us vs the 462us 2-slot fp32r baseline (tensor ~255us busy,
~241us ideal; ~11us fixed start preamble, ~10us teardown).
"""

import numpy as np
import ml_dtypes
from contextlib import ExitStack

import concourse.bacc as bacc
import concourse.tile as tile
import concourse.mybir as mybir
from concourse.bass_utils import run_bass_kernel_spmd

# problem dims (fixed by the graded problem)
T, D, I, E = 1024, 2048, 1408, 16
SI = 2 * I               # shared expert intermediate (2816)
TOP_K, N_GROUP, TOPK_GROUP = 6, 4, 2
ROUTED_SCALE = 2.5
NCORES = 8
NSLOT = 3                # routed expert slots per core
KT = D // 128            # 16 contraction tiles (gemm1)
IT = I // 128            # 11 contraction tiles (gemm2, routed)
DT = D // 128            # 16 output d-tiles (gemm2)
SSLICE = SI // NCORES    # 352 shared-intermediate rows per core
SIP = 384                # padded to 3x128
SIT = SIP // 128         # 3

f32 = mybir.dt.float32
bf16 = mybir.dt.bfloat16
nbf16 = ml_dtypes.bfloat16
ACT_SILU = mybir.ActivationFunctionType.Silu


# ---------------------------------------------------------------- routing
def _route(x, gate_w, bias):
    """Replicates the jax reference gate in numpy f32 (decision margins are
    >=1e-4 so ulp-level differences cannot flip the top-k).

    Returns topk_idx [T,6] int, weights [T,6] f32 (renormalized, unscaled).
    """
    logits = (x @ gate_w.T).astype(np.float32)
    scores = (1.0 / (1.0 + np.exp(-logits))).astype(np.float32)
    s_choice = scores + bias.astype(np.float32)
    grp = s_choice.reshape(T, N_GROUP, E // N_GROUP)
    group_scores = np.sort(grp, axis=2)[:, :, -2:].sum(2, dtype=np.float32)
    grp_idx = np.argsort(-group_scores, axis=1, kind="stable")[:, :TOPK_GROUP]
    gmask = np.zeros((T, N_GROUP), dtype=bool)
    gmask[np.arange(T)[:, None], grp_idx] = True
    emask = np.repeat(gmask, E // N_GROUP, axis=1)
    masked = np.where(emask, s_choice, -np.inf)
    topk_idx = np.argsort(-masked, axis=1, kind="stable")[:, :TOP_K]
    w = np.take_along_axis(scores, topk_idx, axis=1)
    w = (w / w.sum(axis=1, keepdims=True)).astype(np.float32)
    return topk_idx, w


# ---------------------------------------------------- slot capacity search
def _solve_caps(counts):
    """Pick slot capacities (a >= b >= c, multiples of 16, <= 512) with an
    exact DP feasibility check over the 8x3 cells; an expert may span
    several cells.  Returns (caps, per-expert cell usage [(na, nb, nc)])."""
    from functools import lru_cache
    counts = [int(k) for k in counts]
    ideal = sum(counts) / NCORES

    def solve(a, b, c):
        opts_per = []
        for k in counts:
            opts = []
            for na in range(0, 9):
                if na * a >= k + a and na > 0:
                    break
                for nb in range(0, 9):
                    if na * a + nb * b >= k + b and nb > 0:
                        break
                    for ncc in range(0, 9):
                        if na * a + nb * b + ncc * c >= k:
                            opts.append((na, nb, ncc))
                            break
            if not opts:
                return None
            opts_per.append(opts)

        @lru_cache(maxsize=None)
        def dp(i, ua, ub, uc):
            if i == len(counts):
                return ()
            for na, nb, ncc in opts_per[i]:
                if ua + na <= 8 and ub + nb <= 8 and uc + ncc <= 8:
                    r = dp(i + 1, ua + na, ub + nb, uc + ncc)
                    if r is not None:
                        return ((na, nb, ncc),) + r
            return None
        return dp(0, 0, 0, 0)

    best = None
    for a in range(256, 513, 8):
        for b in range(96, a + 1, 8):
            for c in range(16, b + 1, 8):
                s = a + b + c
                if s < ideal or (best is not None and s >= best[0][0]):
                    continue
                r = solve(a, b, c)
                if r is not None:
                    ncells = sum(sum(o) for o in r)
                    key = (s, ncells)
                    if best is None or key < best[0]:
                        best = (key, (a, b, c), r)
    if best is None:  # fallback: single big slot class per expert (always ok)
        a = min(512, 16 * ((max(counts) + 15) // 16))
        return (a, a, a), solve(a, a, a)
    return best[1], best[2]


# ------------------------------------------------------------ host packing
def _pack_wgu(w, it_cnt):
    """w: [2*ic, D] rows (gate block then up block, ic=128*it_cnt rows each)
    -> bf16 [it_cnt, 128, 2, KT, 128]: per i-tile one gate/up PAIR, loaded
    as a single 8KB-per-partition DMA (halves the weight-queue descriptor
    count); partition dim is the contraction (d)."""
    ic = 128 * it_cnt
    g = w[:ic].reshape(it_cnt, 128, D)
    u = w[ic:].reshape(it_cnt, 128, D)
    inter = np.stack([g, u], axis=1).reshape(2 * it_cnt * 128, D)
    t = inter.T.reshape(KT, 128, 2 * it_cnt, 128).transpose(2, 1, 0, 3)
    # [2*it, 128, KT, 128] -> [it, 2, 128, KT, 128] -> [it, 128, 2, KT, 128]
    t = t.reshape(it_cnt, 2, 128, KT, 128).transpose(0, 2, 1, 3, 4)
    return np.ascontiguousarray(t.astype(nbf16))


def _pack_wdT(wdT, it_cnt):
    """wdT: [128*it_cnt, D] (= w_down^T, zero-padded rows allowed)
    -> bf16 [DT, 128, it_cnt, 128]: stationary tiles [i-part, d] per
    (d-tile, i-k-tile) for the transposed GEMM2."""
    t = wdT.reshape(it_cnt, 128, DT, 128).transpose(2, 1, 0, 3)
    return np.ascontiguousarray(t.astype(nbf16))


def _pack_xT(xs, cap):
    """xs: [n, D] token rows -> bf16 [128, KT, cap] (x^T k-tiles, padded)."""
    out = np.zeros((128, KT, cap), dtype=nbf16)
    n = xs.shape[0]
    if n:
        out[:, :, :n] = xs.T.reshape(KT, 128, n).transpose(1, 0, 2).astype(nbf16)
    return out


# ------------------------------------------------------------ device build
def _build(caps):
    nc = bacc.Bacc("TRN2", target_bir_lowering=False, debug=False,
                   num_devices=NCORES)

    slot_in = []
    for s, c in enumerate(caps):
        slot_in.append(dict(
            xg=nc.dram_tensor(f"xg{s}", [128, KT, c], bf16, kind="ExternalInput"),
            wgu=nc.dram_tensor(f"wgu{s}", [IT, 128, 2, KT, 128], bf16, kind="ExternalInput"),
            wd=nc.dram_tensor(f"wd{s}", [DT, 128, IT, 128], bf16, kind="ExternalInput"),
            cwb=nc.dram_tensor(f"cwb{s}", [128, c], f32, kind="ExternalInput"),
            # output transposed + 4 d-tiles batched: [g][128][j][tok]
            yr=nc.dram_tensor(f"yr{s}", [DT // 4, 128, 4, c], bf16, kind="ExternalOutput"),
        ))
    xt_d = nc.dram_tensor("xt", [128, KT, T], bf16, kind="ExternalInput")
    wsgu_d = nc.dram_tensor("wsgu", [SIT, 128, 2, KT, 128], bf16, kind="ExternalInput")
    wsd_d = nc.dram_tensor("wsd", [128, DT, SIT, 128], bf16, kind="ExternalInput")
    ys_d = nc.dram_tensor("ys", [DT // 4, 2, 128, 4, 512], bf16, kind="ExternalOutput")

    with tile.TileContext(nc) as tc, ExitStack() as ctx:
        sb = ctx.enter_context(tc.tile_pool(name="sb", bufs=1))
        ps = ctx.enter_context(tc.tile_pool(name="ps", bufs=1, space="PSUM"))

        def load_x(xg_d, cap, sfx, eng=None, pieces=2):
            # x^T in `pieces` batched k-range loads so the first matmuls can
            # start while the rest streams in
            eng = eng or nc.gpsimd
            xg = sb.tile([128, KT, cap], bf16, tag=f"xg{sfx}", bufs=1, name=f"xg{sfx}")
            h = KT // pieces
            for q in range(pieces):
                eng.dma_start(xg[:, q * h:(q + 1) * h, :], xg_d.ap()[:, q * h:(q + 1) * h, :])
            return xg

        def gemm1(xat, cap, chunks, it_cnt, wgu_d, tag, hook=None):
            # GEMM1 + silu*mul -> at (A^T, [i, tokens], bf16)
            # xat(k, ci, off, n) yields the [128, n] moving x slice
            at = sb.tile([128, it_cnt, cap], bf16, tag=tag, bufs=1, name=tag)
            for t in range(it_cnt):
                wgu = sb.tile([128, 2, KT, 128], bf16, tag="wgu", bufs=4, name="wgu")
                nc.sync.dma_start(wgu[:], wgu_d.ap()[t])
                pair = []
                for par in (0, 1):
                    row = []
                    for ci, (off, n) in enumerate(chunks):
                        p = ps.tile([128, n], f32, tag=f"ps{par}", bufs=2, name=f"ps{par}")
                        for k in range(KT):
                            nc.tensor.matmul(p[:], wgu[:, par, k, :], xat(k, ci, off, n),
                                             start=(k == 0), stop=(k == KT - 1))
                        row.append(p)
                    pair.append(row)
                if hook is not None:
                    hook(t)
                for ci, (off, n) in enumerate(chunks):
                    tmp = sb.tile([128, n], f32, tag="tmp", bufs=2, name="tmp")
                    nc.scalar.activation(tmp[:], pair[0][ci][:], ACT_SILU)
                    nc.vector.tensor_mul(at[:, t, off:off + n], tmp[:], pair[1][ci][:])
            return at

        def preload_wd(s):
            # w_down tiles stream on the sync queue interleaved with wgu;
            # all 16 resident (45KB/partition) so no consumption gating
            wds = []
            def hook(t):
                while len(wds) < min(2 * (t + 1), DT):
                    wd = sb.tile([128, IT, 128], bf16, tag="wd", bufs=16, name="wd")
                    nc.sync.dma_start(wd[:], slot_in[s]["wd"].ap()[len(wds)])
                    wds.append(wd)
            return wds, hook

        def load_slot_x(s, cap, eng):
            io = slot_in[s]
            xg = load_x(io["xg"], cap, str(s), eng=eng)
            cwb = sb.tile([128, cap], f32, tag=f"cwb{s}", bufs=1, name=f"cwb{s}")
            eng.dma_start(cwb[:], io["cwb"].ap()[:])
            return xg, cwb

        def gemm2_routed(s, cap, at, cwb, wds):
            io = slot_in[s]
            # GEMM2 (transposed): out[d, tok] = w_down^T[i, d]^T @ at[i, tok]
            # 4 consecutive d-tiles batched into one output DMA
            for g in range(DT // 4):
                ysb4 = sb.tile([128, 4, cap], bf16, tag="ysb4", bufs=4, name="ysb4")
                for j in range(4):
                    dt = 4 * g + j
                    yp = ps.tile([128, cap], f32, tag="psy", bufs=4, name="yp")
                    for k in range(IT):
                        nc.tensor.matmul(yp[:], wds[dt][:, k, :], at[:, k, :],
                                         start=(k == 0), stop=(k == IT - 1))
                    nc.vector.tensor_mul(ysb4[:, j, :], yp[:], cwb[:])
                nc.scalar.dma_start(io["yr"].ap()[g], ysb4[:])

        def gemm2_shared(at):
            # all 16 w_down^T tiles fit in one resident tile (12KB/partition)
            wsd = sb.tile([128, DT, SIT, 128], bf16, tag="wsd", bufs=1, name="wsd")
            nc.gpsimd.dma_start(wsd[:], wsd_d.ap()[:])
            for ci, (off, n) in enumerate([(0, 512), (512, 512)]):
                for g in range(DT // 4):
                    ysb4 = sb.tile([128, 4, 512], bf16, tag="ysb4s", bufs=2, name="ysb4s")
                    for j in range(4):
                        dt = 4 * g + j
                        yp = ps.tile([128, 512], f32, tag="psy", bufs=4, name="yp")
                        for k in range(SIT):
                            nc.tensor.matmul(yp[:], wsd[:, dt, k, :], at[:, k, off:off + n],
                                             start=(k == 0), stop=(k == SIT - 1))
                        nc.vector.tensor_copy(ysb4[:, j, :], yp[:])
                    # alternate the two HW queues so the tail drains in parallel
                    eng = nc.sync if ((4 * ci + g) % 2) else nc.scalar
                    eng.dma_start(ys_d.ap()[g, ci], ysb4[:])

        # Phase order: shared GEMM1 runs FIRST as a warm-up curtain — it
        # consumes weights at only ~75 GB/s for ~40us, giving the slot-0
        # weight streams a deep prefetch head start — and shared GEMM2 runs
        # LAST, a weight-free phase under which the output queues drain.
        # All x-side loads ride the scalar HW queue; their triggers sit
        # between the ACT instructions of the preceding GEMM1 phase, which
        # throttles each prefetch to fire one phase ahead of use.
        xt = load_x(xt_d, T, "xts", eng=nc.scalar, pieces=4)
        xg0, cwb0 = load_slot_x(0, caps[0], nc.scalar)
        at_s = gemm1(lambda k, ci, off, n: xt[:, k, off:off + n], T,
                     [(0, 512), (512, 512)], SIT, wsgu_d, "atS")
        def xacc(xg):
            return lambda k, ci, off, n: xg[:, k, off:off + n]

        xg1, cwb1 = load_slot_x(1, caps[1], nc.scalar)
        wds0, hook0 = preload_wd(0)
        at0 = gemm1(xacc(xg0), caps[0], [(0, caps[0])], IT, slot_in[0]["wgu"], "atA", hook0)
        gemm2_routed(0, caps[0], at0, cwb0, wds0)
        xg2, cwb2 = load_slot_x(2, caps[2], nc.scalar)
        wds1, hook1 = preload_wd(1)
        at1 = gemm1(xacc(xg1), caps[1], [(0, caps[1])], IT, slot_in[1]["wgu"], "atB", hook1)
        gemm2_routed(1, caps[1], at1, cwb1, wds1)
        wds2, hook2 = preload_wd(2)
        at2 = gemm1(xacc(xg2), caps[2], [(0, caps[2])], IT, slot_in[2]["wgu"], "atA", hook2)
        gemm2_routed(2, caps[2], at2, cwb2, wds2)
        gemm2_shared(at_s)

    nc.compile()
    return nc


# ----------------------------------------------------------------- kernel
def kernel(x, gate_w, bias, w_gate_up, w_down, shared_w_gate_up,
           shared_w_down, _trace=False):
    x = np.ascontiguousarray(x, dtype=np.float32)
    topk_idx, w = _route(x, gate_w, bias)
    cw_full = w.astype(np.float32) * np.float32(ROUTED_SCALE)

    # expert -> token list + weight list
    toks, wts, counts = [], [], np.zeros(E, dtype=np.int64)
    for e in range(E):
        tsel, ksel = np.where(topk_idx == e)
        toks.append(tsel)
        wts.append(cw_full[tsel, ksel])
        counts[e] = len(tsel)

    caps, usage = _solve_caps(counts)

    # split each expert's tokens into cell pieces; distribute cells to cores
    # cells[s] = list over cores of (expert_id, tok_idx, weights)
    cells = [[] for _ in range(NSLOT)]
    for e in range(E):
        na_nb_nc = usage[e]
        pos = 0
        for s in range(NSLOT):
            for _ in range(na_nb_nc[s]):
                take = min(caps[s], counts[e] - pos)
                if take < 0:
                    take = 0
                cells[s].append((e, toks[e][pos:pos + take], wts[e][pos:pos + take]))
                pos += take
    for s in range(NSLOT):
        assert len(cells[s]) <= NCORES, (caps, [len(c) for c in cells])
        while len(cells[s]) < NCORES:
            cells[s].append((0, np.zeros(0, np.int64), np.zeros(0, np.float32)))

    # pack each expert's weights once (bf16); cells alias these arrays
    packed_wgu = {}
    packed_wd = {}
    for e in set(c[0] for s in range(NSLOT) for c in cells[s]):
        packed_wgu[e] = _pack_wgu(w_gate_up[e], IT)
        packed_wd[e] = _pack_wdT(np.ascontiguousarray(w_down[e].T), IT)

    xt_packed = _pack_xT(x, T)
    in_maps = []
    for core in range(NCORES):
        m = {}
        for s, c in enumerate(caps):
            eid, tsel, tw = cells[s][core]
            m[f"xg{s}"] = _pack_xT(x[tsel], c)
            m[f"wgu{s}"] = packed_wgu[eid]
            m[f"wd{s}"] = packed_wd[eid]
            cwb = np.zeros((128, c), dtype=np.float32)
            cwb[:, :len(tw)] = tw[None, :]
            m[f"cwb{s}"] = cwb
        # shared expert slice (rows [352c, 352c+352), zero-padded to 384)
        gsl = np.zeros((2 * SIP, D), dtype=np.float32)
        gsl[:SSLICE] = shared_w_gate_up[SSLICE * core: SSLICE * (core + 1)]
        gsl[SIP:SIP + SSLICE] = shared_w_gate_up[SI + SSLICE * core: SI + SSLICE * (core + 1)]
        m["wsgu"] = _pack_wgu(gsl, SIT)
        sdT = np.zeros((SIP, D), dtype=np.float32)
        sdT[:SSLICE] = shared_w_down[:, SSLICE * core: SSLICE * (core + 1)].T
        m["wsd"] = np.ascontiguousarray(_pack_wdT(sdT, SIT).transpose(1, 0, 2, 3))
        m["xt"] = xt_packed
        in_maps.append(m)

    nc = _build(caps)
    kw = {}
    if _trace:
        kw = dict(trace=True, trace_cores=list(range(NCORES)))
    res = run_bass_kernel_spmd(nc, in_maps, core_ids=list(range(NCORES)), **kw)

    y = np.zeros((T, D), dtype=np.float32)
    for core in range(NCORES):
        # ys: [g, ci, p, j, u] -> [d = g*512 + j*128 + p, t = ci*512 + u]
        ys = res.results[core]["ys"].astype(np.float32)
        y += ys.transpose(0, 3, 2, 1, 4).reshape(D, T).T
    for core in range(NCORES):
        for s in range(NSLOT):
            eid, tsel, _ = cells[s][core]
            n = len(tsel)
            if n:
                # yr: [g, p, j, tok] -> [d = g*512 + j*128 + p, tok]
                yr = res.results[core][f"yr{s}"].astype(np.float32)
                c = yr.shape[-1]
                y[tsel] += yr.transpose(0, 2, 1, 3).reshape(D, c)[:, :n].T
    if _trace:
        return y, res
    return y


# revision 43
# speedup vs baseline: 1.0287x; 1.0287x over previous
"""DeepseekV2 MoE layer on 8 Trainium2 NeuronCores (Bass/Tile, SPMD).

Strategy (expert-parallel with split-expert load balancing, bf16 matmuls):
 - Host computes the MoE gate routing in numpy (matches the jax reference:
   top-k margins are ~1e-4, far above ulp noise).
 - Routed experts run in S=3 uniform "slots" per core (SPMD needs uniform
   shapes).  Slot capacities (C0 >= C1 >= C2) are chosen by a small exact
   DP so that the 24 cells (8 cores x 3 slots) can hold all 16 experts,
   splitting a large expert's tokens across several cells.  This cuts the
   per-core padded token count from ~1040 (2-slot scheme) to ~816.
 - All matmul operands are bf16 (full PE rate, half the HBM traffic of
   fp32; rel err ~5e-3 vs the 2e-2 gate).  PSUM accumulates fp32.
 - GEMM2 is computed transposed: stationary = w_down^T tiles, moving =
   activations [i, tokens].  Output lands as [D, cap] (host transposes),
   so compute scales exactly with cap and the combine-weight scale is a
   single fused psum->sbuf multiply against a host-broadcast [128, cap]
   weight tile.
 - Shared expert is TP-sharded over its intermediate dim (352 rows per
   core, padded to 384), same transposed-GEMM2 scheme, summed on host.

The schedule is DMA/compute balanced (per core ~72MB at ~300GB/s vs
~241us of matmul rows), so queue placement is what the tuning is about:
 - sync HW DGE queue: wgu stream + all 16 w_down tiles per slot,
   interleaved via a hook inside GEMM1 (no consumption gating).
 - scalar HW DGE queue: x-side prefetch (throttled one phase ahead by the
   ACT instructions that sit between the triggers) + batched outputs.
 - gpsimd SW DGE queue (1us/transfer overhead): only the single resident
   shared-w_down transfer.
 - Shared-expert GEMM1 runs FIRST (weight-light warm-up curtain: deep
   slot-0 prefetch + HAM clock ramp), shared GEMM2 LAST (weight-free
   phase under which the output queues drain, split across both HW
   queues).
Measured: 283us vs the 462us 2-slot fp32r baseline (tensor ~255us busy,
~241us ideal; ~11us fixed start preamble, ~10us teardown).
"""

import numpy as np
import ml_dtypes
from contextlib import ExitStack

import concourse.bacc as bacc
import concourse.tile as tile
import concourse.mybir as mybir
from concourse.bass_utils import run_bass_kernel_spmd

# problem dims (fixed by the graded problem)
T, D, I, E = 1024, 2048, 1408, 16
SI = 2 * I               # shared expert intermediate (2816)
TOP_K, N_GROUP, TOPK_GROUP = 6, 4, 2
ROUTED_SCALE = 2.5
NCORES = 8
NSLOT = 3                # routed expert slots per core
KT = D // 128            # 16 contraction tiles (gemm1)
IT = I // 128            # 11 contraction tiles (gemm2, routed)
DT = D // 128            # 16 output d-tiles (gemm2)
SSLICE = SI // NCORES    # 352 shared-intermediate rows per core
SIP = 384                # padded to 3x128
SIT = SIP // 128         # 3

f32 = mybir.dt.float32
bf16 = mybir.dt.bfloat16
nbf16 = ml_dtypes.bfloat16
ACT_SILU = mybir.ActivationFunctionType.Silu


# ---------------------------------------------------------------- routing
def _route(x, gate_w, bias):
    """Replicates the jax reference gate in numpy f32 (decision margins are
    >=1e-4 so ulp-level differences cannot flip the top-k).

    Returns topk_idx [T,6] int, weights [T,6] f32 (renormalized, unscaled).
    """
    logits = (x @ gate_w.T).astype(np.float32)
    scores = (1.0 / (1.0 + np.exp(-logits))).astype(np.float32)
    s_choice = scores + bias.astype(np.float32)
    grp = s_choice.reshape(T, N_GROUP, E // N_GROUP)
    group_scores = np.sort(grp, axis=2)[:, :, -2:].sum(2, dtype=np.float32)
    grp_idx = np.argsort(-group_scores, axis=1, kind="stable")[:, :TOPK_GROUP]
    gmask = np.zeros((T, N_GROUP), dtype=bool)
    gmask[np.arange(T)[:, None], grp_idx] = True
    emask = np.repeat(gmask, E // N_GROUP, axis=1)
    masked = np.where(emask, s_choice, -np.inf)
    topk_idx = np.argsort(-masked, axis=1, kind="stable")[:, :TOP_K]
    w = np.take_along_axis(scores, topk_idx, axis=1)
    w = (w / w.sum(axis=1, keepdims=True)).astype(np.float32)
    return topk_idx, w


# ---------------------------------------------------- slot capacity search
def _solve_caps(counts):
    """Pick slot capacities (a >= b >= c, multiples of 16, <= 512) with an
    exact DP feasibility check over the 8x3 cells; an expert may span
    several cells.  Returns (caps, per-expert cell usage [(na, nb, nc)])."""
    from functools import lru_cache
    counts = [int(k) for k in counts]
    ideal = sum(counts) / NCORES

    def solve(a, b, c):
        opts_per = []
        for k in counts:
            opts = []
            for na in range(0, 9):
                if na * a >= k + a and na > 0:
                    break
                for nb in range(0, 9):
                    if na * a + nb * b >= k + b and nb > 0:
                        break
                    for ncc in range(0, 9):
                        if na * a + nb * b + ncc * c >= k:
                            opts.append((na, nb, ncc))
                            break
            if not opts:
                return None
            opts_per.append(opts)

        @lru_cache(maxsize=None)
        def dp(i, ua, ub, uc):
            if i == len(counts):
                return ()
            for na, nb, ncc in opts_per[i]:
                if ua + na <= 8 and ub + nb <= 8 and uc + ncc <= 8:
                    r = dp(i + 1, ua + na, ub + nb, uc + ncc)
                    if r is not None:
                        return ((na, nb, ncc),) + r
            return None
        return dp(0, 0, 0, 0)

    best = None
    for a in range(256, 513, 8):
        for b in range(96, a + 1, 8):
            for c in range(16, b + 1, 8):
                s = a + b + c
                if s < ideal or (best is not None and s >= best[0][0]):
                    continue
                r = solve(a, b, c)
                if r is not None:
                    ncells = sum(sum(o) for o in r)
                    key = (s, ncells)
                    if best is None or key < best[0]:
                        best = (key, (a, b, c), r)
    if best is None:  # fallback: single big slot class per expert (always ok)
        a = min(512, 16 * ((max(counts) + 15) // 16))
        return (a, a, a), solve(a, a, a)
    return best[1], best[2]


# ------------------------------------------------------------ host packing
def _pack_wgu(w, it_cnt):
    """w: [2*ic, D] rows (gate block then up block, ic=128*it_cnt rows each)
    -> bf16 [2*it_cnt, 128, KT, 128] with gate/up 128-row tiles interleaved;
    tile m is w^T[k-tile, m-tile] with partition dim = contraction (d)."""
    ic = 128 * it_cnt
    g = w[:ic].reshape(it_cnt, 128, D)
    u = w[ic:].reshape(it_cnt, 128, D)
    inter = np.stack([g, u], axis=1).reshape(2 * it_cnt * 128, D)
    t = inter.T.reshape(KT, 128, 2 * it_cnt, 128).transpose(2, 1, 0, 3)
    return np.ascontiguousarray(t.astype(nbf16))


def _pack_wdT(wdT, it_cnt):
    """wdT: [128*it_cnt, D] (= w_down^T, zero-padded rows allowed)
    -> bf16 [DT, 128, it_cnt, 128]: stationary tiles [i-part, d] per
    (d-tile, i-k-tile) for the transposed GEMM2."""
    t = wdT.reshape(it_cnt, 128, DT, 128).transpose(2, 1, 0, 3)
    return np.ascontiguousarray(t.astype(nbf16))


def _pack_xT(xs, cap):
    """xs: [n, D] token rows -> bf16 [128, KT, cap] (x^T k-tiles, padded)."""
    out = np.zeros((128, KT, cap), dtype=nbf16)
    n = xs.shape[0]
    if n:
        out[:, :, :n] = xs.T.reshape(KT, 128, n).transpose(1, 0, 2).astype(nbf16)
    return out


# ------------------------------------------------------------ device build
def _build(caps):
    nc = bacc.Bacc("TRN2", target_bir_lowering=False, debug=False,
                   num_devices=NCORES)

    slot_in = []
    for s, c in enumerate(caps):
        slot_in.append(dict(
            xg=nc.dram_tensor(f"xg{s}", [128, KT, c], bf16, kind="ExternalInput"),
            wgu=nc.dram_tensor(f"wgu{s}", [2 * IT, 128, KT, 128], bf16, kind="ExternalInput"),
            wd=nc.dram_tensor(f"wd{s}", [DT, 128, IT, 128], bf16, kind="ExternalInput"),
            cwb=nc.dram_tensor(f"cwb{s}", [128, c], f32, kind="ExternalInput"),
            # output transposed + 4 d-tiles batched: [g][128][j][tok]
            yr=nc.dram_tensor(f"yr{s}", [DT // 4, 128, 4, c], bf16, kind="ExternalOutput"),
        ))
    xt_d = nc.dram_tensor("xt", [128, KT, T], bf16, kind="ExternalInput")
    wsgu_d = nc.dram_tensor("wsgu", [2 * SIT, 128, KT, 128], bf16, kind="ExternalInput")
    wsd_d = nc.dram_tensor("wsd", [128, DT, SIT, 128], bf16, kind="ExternalInput")
    ys_d = nc.dram_tensor("ys", [DT // 4, 2, 128, 4, 512], bf16, kind="ExternalOutput")

    with tile.TileContext(nc) as tc, ExitStack() as ctx:
        sb = ctx.enter_context(tc.tile_pool(name="sb", bufs=1))
        ps = ctx.enter_context(tc.tile_pool(name="ps", bufs=1, space="PSUM"))

        def load_x(xg_d, cap, sfx, eng=None, pieces=2):
            # x^T in `pieces` batched k-range loads so the first matmuls can
            # start while the rest streams in
            eng = eng or nc.gpsimd
            xg = sb.tile([128, KT, cap], bf16, tag=f"xg{sfx}", bufs=1, name=f"xg{sfx}")
            h = KT // pieces
            for q in range(pieces):
                eng.dma_start(xg[:, q * h:(q + 1) * h, :], xg_d.ap()[:, q * h:(q + 1) * h, :])
            return xg

        def gemm1(xat, cap, chunks, it_cnt, wgu_d, tag, hook=None):
            # GEMM1 + silu*mul -> at (A^T, [i, tokens], bf16)
            # xat(k, ci, off, n) yields the [128, n] moving x slice
            at = sb.tile([128, it_cnt, cap], bf16, tag=tag, bufs=1, name=tag)
            for t in range(it_cnt):
                pair = []
                for par in (0, 1):
                    wgu = sb.tile([128, KT, 128], bf16, tag="wgu", bufs=8, name="wgu")
                    nc.sync.dma_start(wgu[:], wgu_d.ap()[2 * t + par])
                    row = []
                    for ci, (off, n) in enumerate(chunks):
                        p = ps.tile([128, n], f32, tag=f"ps{par}", bufs=2, name=f"ps{par}")
                        for k in range(KT):
                            nc.tensor.matmul(p[:], wgu[:, k, :], xat(k, ci, off, n),
                                             start=(k == 0), stop=(k == KT - 1))
                        row.append(p)
                    pair.append(row)
                if hook is not None:
                    hook(t)
                for ci, (off, n) in enumerate(chunks):
                    tmp = sb.tile([128, n], f32, tag="tmp", bufs=2, name="tmp")
                    nc.scalar.activation(tmp[:], pair[0][ci][:], ACT_SILU)
                    nc.vector.tensor_mul(at[:, t, off:off + n], tmp[:], pair[1][ci][:])
            return at

        def preload_wd(s):
            # w_down tiles stream on the sync queue interleaved with wgu;
            # all 16 resident (45KB/partition) so no consumption gating
            wds = []
            def hook(t):
                while len(wds) < min(2 * (t + 1), DT):
                    wd = sb.tile([128, IT, 128], bf16, tag="wd", bufs=16, name="wd")
                    nc.sync.dma_start(wd[:], slot_in[s]["wd"].ap()[len(wds)])
                    wds.append(wd)
            return wds, hook

        def load_slot_x(s, cap, eng):
            io = slot_in[s]
            xg = load_x(io["xg"], cap, str(s), eng=eng)
            cwb = sb.tile([128, cap], f32, tag=f"cwb{s}", bufs=1, name=f"cwb{s}")
            eng.dma_start(cwb[:], io["cwb"].ap()[:])
            return xg, cwb

        def gemm2_routed(s, cap, at, cwb, wds):
            io = slot_in[s]
            # GEMM2 (transposed): out[d, tok] = w_down^T[i, d]^T @ at[i, tok]
            # 4 consecutive d-tiles batched into one output DMA
            for g in range(DT // 4):
                ysb4 = sb.tile([128, 4, cap], bf16, tag="ysb4", bufs=4, name="ysb4")
                for j in range(4):
                    dt = 4 * g + j
                    yp = ps.tile([128, cap], f32, tag="psy", bufs=4, name="yp")
                    for k in range(IT):
                        nc.tensor.matmul(yp[:], wds[dt][:, k, :], at[:, k, :],
                                         start=(k == 0), stop=(k == IT - 1))
                    nc.vector.tensor_mul(ysb4[:, j, :], yp[:], cwb[:])
                nc.scalar.dma_start(io["yr"].ap()[g], ysb4[:])

        def gemm2_shared(at):
            # all 16 w_down^T tiles fit in one resident tile (12KB/partition)
            wsd = sb.tile([128, DT, SIT, 128], bf16, tag="wsd", bufs=1, name="wsd")
            nc.gpsimd.dma_start(wsd[:], wsd_d.ap()[:])
            for ci, (off, n) in enumerate([(0, 512), (512, 512)]):
                for g in range(DT // 4):
                    ysb4 = sb.tile([128, 4, 512], bf16, tag="ysb4s", bufs=2, name="ysb4s")
                    for j in range(4):
                        dt = 4 * g + j
                        yp = ps.tile([128, 512], f32, tag="psy", bufs=4, name="yp")
                        for k in range(SIT):
                            nc.tensor.matmul(yp[:], wsd[:, dt, k, :], at[:, k, off:off + n],
                                             start=(k == 0), stop=(k == SIT - 1))
                        nc.vector.tensor_copy(ysb4[:, j, :], yp[:])
                    # alternate the two HW queues so the tail drains in parallel
                    eng = nc.sync if ((4 * ci + g) % 2) else nc.scalar
                    eng.dma_start(ys_d.ap()[g, ci], ysb4[:])

        # Phase order: shared GEMM1 runs FIRST as a warm-up curtain — it
        # consumes weights at only ~75 GB/s for ~40us, giving the slot-0
        # weight streams a deep prefetch head start — and shared GEMM2 runs
        # LAST, a weight-free phase under which the output queues drain.
        # All x-side loads ride the scalar HW queue; their triggers sit
        # between the ACT instructions of the preceding GEMM1 phase, which
        # throttles each prefetch to fire one phase ahead of use.
        xt = load_x(xt_d, T, "xts", eng=nc.scalar, pieces=4)
        xg0, cwb0 = load_slot_x(0, caps[0], nc.scalar)
        at_s = gemm1(lambda k, ci, off, n: xt[:, k, off:off + n], T,
                     [(0, 512), (512, 512)], SIT, wsgu_d, "atS")
        def xacc(xg):
            return lambda k, ci, off, n: xg[:, k, off:off + n]

        xg1, cwb1 = load_slot_x(1, caps[1], nc.scalar)
        wds0, hook0 = preload_wd(0)
        at0 = gemm1(xacc(xg0), caps[0], [(0, caps[0])], IT, slot_in[0]["wgu"], "atA", hook0)
        gemm2_routed(0, caps[0], at0, cwb0, wds0)
        xg2, cwb2 = load_slot_x(2, caps[2], nc.scalar)
        wds1, hook1 = preload_wd(1)
        at1 = gemm1(xacc(xg1), caps[1], [(0, caps[1])], IT, slot_in[1]["wgu"], "atB", hook1)
        gemm2_routed(1, caps[1], at1, cwb1, wds1)
        wds2, hook2 = preload_wd(2)
        at2 = gemm1(xacc(xg2), caps[2], [(0, caps[2])], IT, slot_in[2]["wgu"], "atA", hook2)
        gemm2_routed(2, caps[2], at2, cwb2, wds2)
        gemm2_shared(at_s)

    nc.compile()
    return nc


# ----------------------------------------------------------------- kernel
def kernel(x, gate_w, bias, w_gate_up, w_down, shared_w_gate_up,
           shared_w_down, _trace=False):
    x = np.ascontiguousarray(x, dtype=np.float32)
    topk_idx, w = _route(x, gate_w, bias)
    cw_full = w.astype(np.float32) * np.float32(ROUTED_SCALE)

    # expert -> token list + weight list
    toks, wts, counts = [], [], np.zeros(E, dtype=np.int64)
    for e in range(E):
        tsel, ksel = np.where(topk_idx == e)
        toks.append(tsel)
        wts.append(cw_full[tsel, ksel])
        counts[e] = len(tsel)

    caps, usage = _solve_caps(counts)

    # split each expert's tokens into cell pieces; distribute cells to cores
    # cells[s] = list over cores of (expert_id, tok_idx, weights)
    cells = [[] for _ in range(NSLOT)]
    for e in range(E):
        na_nb_nc = usage[e]
        pos = 0
        for s in range(NSLOT):
            for _ in range(na_nb_nc[s]):
                take = min(caps[s], counts[e] - pos)
                if take < 0:
                    take = 0
                cells[s].append((e, toks[e][pos:pos + take], wts[e][pos:pos + take]))
                pos += take
    for s in range(NSLOT):
        assert len(cells[s]) <= NCORES, (caps, [len(c) for c in cells])
        while len(cells[s]) < NCORES:
            cells[s].append((0, np.zeros(0, np.int64), np.zeros(0, np.float32)))

    # pack each expert's weights once (bf16); cells alias these arrays
    packed_wgu = {}
    packed_wd = {}
    for e in set(c[0] for s in range(NSLOT) for c in cells[s]):
        packed_wgu[e] = _pack_wgu(w_gate_up[e], IT)
        packed_wd[e] = _pack_wdT(np.ascontiguousarray(w_down[e].T), IT)

    xt_packed = _pack_xT(x, T)
    in_maps = []
    for core in range(NCORES):
        m = {}
        for s, c in enumerate(caps):
            eid, tsel, tw = cells[s][core]
            m[f"xg{s}"] = _pack_xT(x[tsel], c)
            m[f"wgu{s}"] = packed_wgu[eid]
            m[f"wd{s}"] = packed_wd[eid]
            cwb = np.zeros((128, c), dtype=np.float32)
            cwb[:, :len(tw)] = tw[None, :]
            m[f"cwb{s}"] = cwb
        # shared expert slice (rows [352c, 352c+352), zero-padded to 384)
        gsl = np.zeros((2 * SIP, D), dtype=np.float32)
        gsl[:SSLICE] = shared_w_gate_up[SSLICE * core: SSLICE * (core + 1)]
        gsl[SIP:SIP + SSLICE] = shared_w_gate_up[SI + SSLICE * core: SI + SSLICE * (core + 1)]
        m["wsgu"] = _pack_wgu(gsl, SIT)
        sdT = np.zeros((SIP, D), dtype=np.float32)
        sdT[:SSLICE] = shared_w_down[:, SSLICE * core: SSLICE * (core + 1)].T
        m["wsd"] = np.ascontiguousarray(_pack_wdT(sdT, SIT).transpose(1, 0, 2, 3))
        m["xt"] = xt_packed
        in_maps.append(m)

    nc = _build(caps)
    kw = {}
    if _trace:
        kw = dict(trace=True, trace_cores=list(range(NCORES)))
    res = run_bass_kernel_spmd(nc, in_maps, core_ids=list(range(NCORES)), **kw)

    y = np.zeros((T, D), dtype=np.float32)
    for core in range(NCORES):
        # ys: [g, ci, p, j, u] -> [d = g*512 + j*128 + p, t = ci*512 + u]
        ys = res.results[core]["ys"].astype(np.float32)
        y += ys.transpose(0, 3, 2, 1, 4).reshape(D, T).T
    for core in range(NCORES):
        for s in range(NSLOT):
            eid, tsel, _ = cells[s][core]
            n = len(tsel)
            if n:
                # yr: [g, p, j, tok] -> [d = g*512 + j*128 + p, tok]
                yr = res.results[core][f"yr{s}"].astype(np.float32)
                c = yr.shape[-1]
                y[tsel] += yr.transpose(0, 2, 1, 3).reshape(D, c)[:, :n].T
    if _trace:
        return y, res
    return y


# revision 44
# speedup vs baseline: 1.0336x; 1.0048x over previous
"""DeepseekV2 MoE layer on 8 Trainium2 NeuronCores (Bass/Tile, SPMD).

Strategy (expert-parallel with split-expert load balancing, bf16 matmuls):
 - Host computes the MoE gate routing in numpy (matches the jax reference:
   top-k margins are ~1e-4, far above ulp noise).
 - Routed experts run in S=3 uniform "slots" per core (SPMD needs uniform
   shapes).  Slot capacities (C0 >= C1 >= C2) are chosen by a small exact
   DP so that the 24 cells (8 cores x 3 slots) can hold all 16 experts,
   splitting a large expert's tokens across several cells.  This cuts the
   per-core padded token count from ~1040 (2-slot scheme) to ~816.
 - All matmul operands are bf16 (full PE rate, half the HBM traffic of
   fp32; rel err ~5e-3 vs the 2e-2 gate).  PSUM accumulates fp32.
 - GEMM2 is computed transposed: stationary = w_down^T tiles, moving =
   activations [i, tokens].  Output lands as [D, cap] (host transposes),
   so compute scales exactly with cap and the combine-weight scale is a
   single fused psum->sbuf multiply against a host-broadcast [128, cap]
   weight tile.
 - Shared expert is TP-sharded over its intermediate dim (352 rows per
   core, padded to 384), same transposed-GEMM2 scheme, summed on host.

The schedule is DMA/compute balanced (per core ~72MB at ~300GB/s vs
~241us of matmul rows), so queue placement is what the tuning is about:
 - sync HW DGE queue: wgu stream + all 16 w_down tiles per slot,
   interleaved via a hook inside GEMM1 (no consumption gating).
 - scalar HW DGE queue: x-side prefetch (throttled one phase ahead by the
   ACT instructions that sit between the triggers) + batched outputs.
 - gpsimd SW DGE queue (1us/transfer overhead): only the single resident
   shared-w_down transfer.
 - Shared-expert GEMM1 runs FIRST (weight-light warm-up curtain: deep
   slot-0 prefetch + HAM clock ramp), shared GEMM2 LAST (weight-free
   phase under which the output queues drain, split across both HW
   queues).
Measured: 283us vs the 462us 2-slot fp32r baseline (tensor ~255us busy,
~241us ideal; ~11us fixed start preamble, ~10us teardown).
"""

import numpy as np
import ml_dtypes
from contextlib import ExitStack

import concourse.bacc as bacc
import concourse.tile as tile
import concourse.mybir as mybir
from concourse.bass_utils import run_bass_kernel_spmd

# problem dims (fixed by the graded problem)
T, D, I, E = 1024, 2048, 1408, 16
SI = 2 * I               # shared expert intermediate (2816)
TOP_K, N_GROUP, TOPK_GROUP = 6, 4, 2
ROUTED_SCALE = 2.5
NCORES = 8
NSLOT = 3                # routed expert slots per core
KT = D // 128            # 16 contraction tiles (gemm1)
IT = I // 128            # 11 contraction tiles (gemm2, routed)
DT = D // 128            # 16 output d-tiles (gemm2)
SSLICE = SI // NCORES    # 352 shared-intermediate rows per core
SIP = 384                # padded to 3x128
SIT = SIP // 128         # 3

f32 = mybir.dt.float32
bf16 = mybir.dt.bfloat16
nbf16 = ml_dtypes.bfloat16
ACT_SILU = mybir.ActivationFunctionType.Silu


# ---------------------------------------------------------------- routing
def _route(x, gate_w, bias):
    """Replicates the jax reference gate in numpy f32 (decision margins are
    >=1e-4 so ulp-level differences cannot flip the top-k).

    Returns topk_idx [T,6] int, weights [T,6] f32 (renormalized, unscaled).
    """
    logits = (x @ gate_w.T).astype(np.float32)
    scores = (1.0 / (1.0 + np.exp(-logits))).astype(np.float32)
    s_choice = scores + bias.astype(np.float32)
    grp = s_choice.reshape(T, N_GROUP, E // N_GROUP)
    group_scores = np.sort(grp, axis=2)[:, :, -2:].sum(2, dtype=np.float32)
    grp_idx = np.argsort(-group_scores, axis=1, kind="stable")[:, :TOPK_GROUP]
    gmask = np.zeros((T, N_GROUP), dtype=bool)
    gmask[np.arange(T)[:, None], grp_idx] = True
    emask = np.repeat(gmask, E // N_GROUP, axis=1)
    masked = np.where(emask, s_choice, -np.inf)
    topk_idx = np.argsort(-masked, axis=1, kind="stable")[:, :TOP_K]
    w = np.take_along_axis(scores, topk_idx, axis=1)
    w = (w / w.sum(axis=1, keepdims=True)).astype(np.float32)
    return topk_idx, w


# ---------------------------------------------------- slot capacity search
def _solve_caps(counts):
    """Pick slot capacities (a >= b >= c, multiples of 16, <= 512) with an
    exact DP feasibility check over the 8x3 cells; an expert may span
    several cells.  Returns (caps, per-expert cell usage [(na, nb, nc)])."""
    from functools import lru_cache
    counts = [int(k) for k in counts]
    ideal = sum(counts) / NCORES

    def solve(a, b, c):
        opts_per = []
        for k in counts:
            opts = []
            for na in range(0, 9):
                if na * a >= k + a and na > 0:
                    break
                for nb in range(0, 9):
                    if na * a + nb * b >= k + b and nb > 0:
                        break
                    for ncc in range(0, 9):
                        if na * a + nb * b + ncc * c >= k:
                            opts.append((na, nb, ncc))
                            break
            if not opts:
                return None
            opts_per.append(opts)

        @lru_cache(maxsize=None)
        def dp(i, ua, ub, uc):
            if i == len(counts):
                return ()
            for na, nb, ncc in opts_per[i]:
                if ua + na <= 8 and ub + nb <= 8 and uc + ncc <= 8:
                    r = dp(i + 1, ua + na, ub + nb, uc + ncc)
                    if r is not None:
                        return ((na, nb, ncc),) + r
            return None
        return dp(0, 0, 0, 0)

    best = None
    for a in range(256, 513, 8):
        for b in range(96, a + 1, 8):
            for c in range(16, b + 1, 8):
                s = a + b + c
                if s < ideal or (best is not None and s >= best[0][0]):
                    continue
                r = solve(a, b, c)
                if r is not None:
                    ncells = sum(sum(o) for o in r)
                    key = (s, ncells)
                    if best is None or key < best[0]:
                        best = (key, (a, b, c), r)
    if best is None:  # fallback: single big slot class per expert (always ok)
        a = min(512, 16 * ((max(counts) + 15) // 16))
        return (a, a, a), solve(a, a, a)
    return best[1], best[2]


# ------------------------------------------------------------ host packing
def _pack_wgu(w, it_cnt):
    """w: [2*ic, D] rows (gate block then up block, ic=128*it_cnt rows each)
    -> bf16 [2*it_cnt, 128, KT, 128] with gate/up 128-row tiles interleaved;
    tile m is w^T[k-tile, m-tile] with partition dim = contraction (d)."""
    ic = 128 * it_cnt
    g = w[:ic].reshape(it_cnt, 128, D)
    u = w[ic:].reshape(it_cnt, 128, D)
    inter = np.stack([g, u], axis=1).reshape(2 * it_cnt * 128, D)
    t = inter.T.reshape(KT, 128, 2 * it_cnt, 128).transpose(2, 1, 0, 3)
    return np.ascontiguousarray(t.astype(nbf16))


def _pack_wdT(wdT, it_cnt):
    """wdT: [128*it_cnt, D] (= w_down^T, zero-padded rows allowed)
    -> bf16 [DT, 128, it_cnt, 128]: stationary tiles [i-part, d] per
    (d-tile, i-k-tile) for the transposed GEMM2."""
    t = wdT.reshape(it_cnt, 128, DT, 128).transpose(2, 1, 0, 3)
    return np.ascontiguousarray(t.astype(nbf16))


def _pack_xT(xs, cap):
    """xs: [n, D] token rows -> bf16 [128, KT, cap] (x^T k-tiles, padded)."""
    out = np.zeros((128, KT, cap), dtype=nbf16)
    n = xs.shape[0]
    if n:
        out[:, :, :n] = xs.T.reshape(KT, 128, n).transpose(1, 0, 2).astype(nbf16)
    return out


# ------------------------------------------------------------ device build
def _build(caps):
    nc = bacc.Bacc("TRN2", target_bir_lowering=False, debug=False,
                   num_devices=NCORES)

    slot_in = []
    for s, c in enumerate(caps):
        slot_in.append(dict(
            xg=nc.dram_tensor(f"xg{s}", [128, KT, c], bf16, kind="ExternalInput"),
            wgu=nc.dram_tensor(f"wgu{s}", [2 * IT, 128, KT, 128], bf16, kind="ExternalInput"),
            wd=nc.dram_tensor(f"wd{s}", [DT, 128, IT, 128], bf16, kind="ExternalInput"),
            cwb=nc.dram_tensor(f"cwb{s}", [128, c], f32, kind="ExternalInput"),
            # output transposed + 4 d-tiles batched: [g][128][j][tok]
            yr=nc.dram_tensor(f"yr{s}", [DT // 4, 128, 4, c], bf16, kind="ExternalOutput"),
        ))
    xt_d = nc.dram_tensor("xt", [128, KT, T], bf16, kind="ExternalInput")
    wsgu_d = nc.dram_tensor("wsgu", [2 * SIT, 128, KT, 128], bf16, kind="ExternalInput")
    wsd_d = nc.dram_tensor("wsd", [128, DT, SIT, 128], bf16, kind="ExternalInput")
    ys_d = nc.dram_tensor("ys", [DT // 4, 2, 128, 4, 512], bf16, kind="ExternalOutput")

    with tile.TileContext(nc) as tc, ExitStack() as ctx:
        sb = ctx.enter_context(tc.tile_pool(name="sb", bufs=1))
        ps = ctx.enter_context(tc.tile_pool(name="ps", bufs=1, space="PSUM"))

        def load_x(xg_d, cap, sfx, eng=None, pieces=2):
            # x^T in `pieces` batched k-range loads so the first matmuls can
            # start while the rest streams in
            eng = eng or nc.gpsimd
            xg = sb.tile([128, KT, cap], bf16, tag=f"xg{sfx}", bufs=1, name=f"xg{sfx}")
            h = KT // pieces
            for q in range(pieces):
                eng.dma_start(xg[:, q * h:(q + 1) * h, :], xg_d.ap()[:, q * h:(q + 1) * h, :])
            return xg

        def gemm1(xat, cap, chunks, it_cnt, wgu_d, tag, hook=None):
            # GEMM1 + silu*mul -> at (A^T, [i, tokens], bf16)
            # xat(k, ci, off, n) yields the [128, n] moving x slice
            at = sb.tile([128, it_cnt, cap], bf16, tag=tag, bufs=1, name=tag)
            for t in range(it_cnt):
                pair = []
                for par in (0, 1):
                    wgu = sb.tile([128, KT, 128], bf16, tag="wgu", bufs=8, name="wgu")
                    nc.sync.dma_start(wgu[:], wgu_d.ap()[2 * t + par])
                    row = []
                    for ci, (off, n) in enumerate(chunks):
                        p = ps.tile([128, n], f32, tag=f"ps{par}", bufs=2, name=f"ps{par}")
                        for k in range(KT):
                            nc.tensor.matmul(p[:], wgu[:, k, :], xat(k, ci, off, n),
                                             start=(k == 0), stop=(k == KT - 1))
                        row.append(p)
                    pair.append(row)
                if hook is not None:
                    hook(t)
                for ci, (off, n) in enumerate(chunks):
                    tmp = sb.tile([128, n], f32, tag="tmp", bufs=2, name="tmp")
                    nc.scalar.activation(tmp[:], pair[0][ci][:], ACT_SILU)
                    nc.vector.tensor_mul(at[:, t, off:off + n], tmp[:], pair[1][ci][:])
            return at

        def preload_wd(s):
            # w_down tiles stream on the sync queue interleaved with wgu;
            # all 16 resident (45KB/partition) so no consumption gating
            wds = []
            def hook(t):
                while len(wds) < min(2 * (t + 1), DT):
                    wd = sb.tile([128, IT, 128], bf16, tag="wd", bufs=16, name="wd")
                    nc.sync.dma_start(wd[:], slot_in[s]["wd"].ap()[len(wds)])
                    wds.append(wd)
            return wds, hook

        def load_slot_x(s, cap, eng):
            io = slot_in[s]
            xg = load_x(io["xg"], cap, str(s), eng=eng)
            cwb = sb.tile([128, cap], f32, tag=f"cwb{s}", bufs=1, name=f"cwb{s}")
            eng.dma_start(cwb[:], io["cwb"].ap()[:])
            return xg, cwb

        def gemm2_routed(s, cap, at, cwb, wds):
            io = slot_in[s]
            # GEMM2 (transposed): out[d, tok] = w_down^T[i, d]^T @ at[i, tok]
            # 4 consecutive d-tiles batched into one output DMA
            for g in range(DT // 4):
                ysb4 = sb.tile([128, 4, cap], bf16, tag="ysb4", bufs=4, name="ysb4")
                for j in range(4):
                    dt = 4 * g + j
                    yp = ps.tile([128, cap], f32, tag="psy", bufs=4, name="yp")
                    for k in range(IT):
                        nc.tensor.matmul(yp[:], wds[dt][:, k, :], at[:, k, :],
                                         start=(k == 0), stop=(k == IT - 1))
                    nc.vector.tensor_mul(ysb4[:, j, :], yp[:], cwb[:])
                nc.scalar.dma_start(io["yr"].ap()[g], ysb4[:])

        def gemm2_shared(at, wsd):
            for ci, (off, n) in enumerate([(0, 512), (512, 512)]):
                for g in range(DT // 4):
                    ysb4 = sb.tile([128, 4, 512], bf16, tag="ysb4s", bufs=2, name="ysb4s")
                    for j in range(4):
                        dt = 4 * g + j
                        yp = ps.tile([128, 512], f32, tag="psy", bufs=4, name="yp")
                        for k in range(SIT):
                            nc.tensor.matmul(yp[:], wsd[:, dt, k, :], at[:, k, off:off + n],
                                             start=(k == 0), stop=(k == SIT - 1))
                        nc.vector.tensor_copy(ysb4[:, j, :], yp[:])
                    # alternate the two HW queues so the tail drains in parallel
                    eng = nc.sync if ((4 * ci + g) % 2) else nc.scalar
                    eng.dma_start(ys_d.ap()[g, ci], ysb4[:])

        # Phase order: shared GEMM1 runs FIRST as a warm-up curtain — it
        # consumes weights at only ~75 GB/s for ~40us, giving the slot-0
        # weight streams a deep prefetch head start — and shared GEMM2 runs
        # LAST, a weight-free phase under which the output queues drain.
        # All x-side loads ride the scalar HW queue; their triggers sit
        # between the ACT instructions of the preceding GEMM1 phase, which
        # throttles each prefetch to fire one phase ahead of use.
        xt = load_x(xt_d, T, "xts", eng=nc.scalar, pieces=4)
        xg0, cwb0 = load_slot_x(0, caps[0], nc.scalar)
        at_s = gemm1(lambda k, ci, off, n: xt[:, k, off:off + n], T,
                     [(0, 512), (512, 512)], SIT, wsgu_d, "atS")
        def xacc(xg):
            return lambda k, ci, off, n: xg[:, k, off:off + n]

        xg1, cwb1 = load_slot_x(1, caps[1], nc.scalar)
        wds0, hook0 = preload_wd(0)
        at0 = gemm1(xacc(xg0), caps[0], [(0, caps[0])], IT, slot_in[0]["wgu"], "atA", hook0)
        gemm2_routed(0, caps[0], at0, cwb0, wds0)
        # shared w_down (all 16 tiles resident, 12KB/partition): emitted here
        # so its trigger fires ~125us in, after slot-0's outputs — NOT at t=0
        # where it would steal bandwidth from the critical xt/wsgu path
        wsd = sb.tile([128, DT, SIT, 128], bf16, tag="wsd", bufs=1, name="wsd")
        nc.scalar.dma_start(wsd[:], wsd_d.ap()[:])
        xg2, cwb2 = load_slot_x(2, caps[2], nc.scalar)
        wds1, hook1 = preload_wd(1)
        at1 = gemm1(xacc(xg1), caps[1], [(0, caps[1])], IT, slot_in[1]["wgu"], "atB", hook1)
        gemm2_routed(1, caps[1], at1, cwb1, wds1)
        wds2, hook2 = preload_wd(2)
        at2 = gemm1(xacc(xg2), caps[2], [(0, caps[2])], IT, slot_in[2]["wgu"], "atA", hook2)
        gemm2_routed(2, caps[2], at2, cwb2, wds2)
        gemm2_shared(at_s, wsd)

    nc.compile()
    return nc


# ----------------------------------------------------------------- kernel
def kernel(x, gate_w, bias, w_gate_up, w_down, shared_w_gate_up,
           shared_w_down, _trace=False):
    x = np.ascontiguousarray(x, dtype=np.float32)
    topk_idx, w = _route(x, gate_w, bias)
    cw_full = w.astype(np.float32) * np.float32(ROUTED_SCALE)

    # expert -> token list + weight list
    toks, wts, counts = [], [], np.zeros(E, dtype=np.int64)
    for e in range(E):
        tsel, ksel = np.where(topk_idx == e)
        toks.append(tsel)
        wts.append(cw_full[tsel, ksel])
        counts[e] = len(tsel)

    caps, usage = _solve_caps(counts)

    # split each expert's tokens into cell pieces; distribute cells to cores
    # cells[s] = list over cores of (expert_id, tok_idx, weights)
    cells = [[] for _ in range(NSLOT)]
    for e in range(E):
        na_nb_nc = usage[e]
        pos = 0
        for s in range(NSLOT):
            for _ in range(na_nb_nc[s]):
                take = min(caps[s], counts[e] - pos)
                if take < 0:
                    take = 0
                cells[s].append((e, toks[e][pos:pos + take], wts[e][pos:pos + take]))
                pos += take
    for s in range(NSLOT):
        assert len(cells[s]) <= NCORES, (caps, [len(c) for c in cells])
        while len(cells[s]) < NCORES:
            cells[s].append((0, np.zeros(0, np.int64), np.zeros(0, np.float32)))

    # pack each expert's weights once (bf16); cells alias these arrays
    packed_wgu = {}
    packed_wd = {}
    for e in set(c[0] for s in range(NSLOT) for c in cells[s]):
        packed_wgu[e] = _pack_wgu(w_gate_up[e], IT)
        packed_wd[e] = _pack_wdT(np.ascontiguousarray(w_down[e].T), IT)

    xt_packed = _pack_xT(x, T)
    in_maps = []
    for core in range(NCORES):
        m = {}
        for s, c in enumerate(caps):
            eid, tsel, tw = cells[s][core]
            m[f"xg{s}"] = _pack_xT(x[tsel], c)
            m[f"wgu{s}"] = packed_wgu[eid]
            m[f"wd{s}"] = packed_wd[eid]
            cwb = np.zeros((128, c), dtype=np.float32)
            cwb[:, :len(tw)] = tw[None, :]
            m[f"cwb{s}"] = cwb
        # shared expert slice (rows [352c, 352c+352), zero-padded to 384)
        gsl = np.zeros((2 * SIP, D), dtype=np.float32)
        gsl[:SSLICE] = shared_w_gate_up[SSLICE * core: SSLICE * (core + 1)]
        gsl[SIP:SIP + SSLICE] = shared_w_gate_up[SI + SSLICE * core: SI + SSLICE * (core + 1)]
        m["wsgu"] = _pack_wgu(gsl, SIT)
        sdT = np.zeros((SIP, D), dtype=np.float32)
        sdT[:SSLICE] = shared_w_down[:, SSLICE * core: SSLICE * (core + 1)].T
        m["wsd"] = np.ascontiguousarray(_pack_wdT(sdT, SIT).transpose(1, 0, 2, 3))
        m["xt"] = xt_packed
        in_maps.append(m)

    nc = _build(caps)
    kw = {}
    if _trace:
        kw = dict(trace=True, trace_cores=list(range(NCORES)))
    res = run_bass_kernel_spmd(nc, in_maps, core_ids=list(range(NCORES)), **kw)

    y = np.zeros((T, D), dtype=np.float32)
    for core in range(NCORES):
        # ys: [g, ci, p, j, u] -> [d = g*512 + j*128 + p, t = ci*512 + u]
        ys = res.results[core]["ys"].astype(np.float32)
        y += ys.transpose(0, 3, 2, 1, 4).reshape(D, T).T
    for core in range(NCORES):
        for s in range(NSLOT):
            eid, tsel, _ = cells[s][core]
            n = len(tsel)
            if n:
                # yr: [g, p, j, tok] -> [d = g*512 + j*128 + p, tok]
                yr = res.results[core][f"yr{s}"].astype(np.float32)
                c = yr.shape[-1]
                y[tsel] += yr.transpose(0, 2, 1, 3).reshape(D, c)[:, :n].T
    if _trace:
        return y, res
    return y


# revision 46
# speedup vs baseline: 1.0379x; 1.0041x over previous
"""DeepseekV2 MoE layer on 8 Trainium2 NeuronCores (Bass/Tile, SPMD).

Strategy (expert-parallel with split-expert load balancing, bf16 matmuls):
 - Host computes the MoE gate routing in numpy (matches the jax reference:
   top-k margins are ~1e-4, far above ulp noise).
 - Routed experts run in S=3 uniform "slots" per core (SPMD needs uniform
   shapes).  Slot capacities (C0 >= C1 >= C2) are chosen by a small exact
   DP so that the 24 cells (8 cores x 3 slots) can hold all 16 experts,
   splitting a large expert's tokens across several cells.  This cuts the
   per-core padded token count from ~1040 (2-slot scheme) to ~816.
 - All matmul operands are bf16 (full PE rate, half the HBM traffic of
   fp32; rel err ~5e-3 vs the 2e-2 gate).  PSUM accumulates fp32.
 - GEMM2 is computed transposed: stationary = w_down^T tiles, moving =
   activations [i, tokens].  Output lands as [D, cap] (host transposes),
   so compute scales exactly with cap and the combine-weight scale is a
   single fused psum->sbuf multiply against a host-broadcast [128, cap]
   weight tile.
 - Shared expert is TP-sharded over its intermediate dim (352 rows per
   core, padded to 384), same transposed-GEMM2 scheme, summed on host.

The schedule is DMA/compute balanced (per core ~72MB at ~300GB/s vs
~241us of matmul rows), so queue placement is what the tuning is about:
 - sync HW DGE queue: wgu stream + all 16 w_down tiles per slot,
   interleaved via a hook inside GEMM1 (no consumption gating).
 - scalar HW DGE queue: x-side prefetch (throttled one phase ahead by the
   ACT instructions that sit between the triggers), the resident
   shared-w_down tile (emitted after slot-0 GEMM2 so it fires ~125us in,
   clear of the critical t=0 xt/wsgu window), and batched outputs.
 - Shared-expert GEMM1 runs FIRST (weight-light warm-up curtain: deep
   slot-0 prefetch + HAM clock ramp), shared GEMM2 LAST (weight-free
   phase under which the output queues drain, split across both HW
   queues).
Measured: ~276us mean / 283us max-core vs the 462us 2-slot fp32r
baseline (tensor ~253us busy, ~241us ideal; ~11us fixed start preamble,
~10us teardown).
"""

import numpy as np
import ml_dtypes
from contextlib import ExitStack

import concourse.bacc as bacc
import concourse.tile as tile
import concourse.mybir as mybir
from concourse.bass_utils import run_bass_kernel_spmd

# problem dims (fixed by the graded problem)
T, D, I, E = 1024, 2048, 1408, 16
SI = 2 * I               # shared expert intermediate (2816)
TOP_K, N_GROUP, TOPK_GROUP = 6, 4, 2
ROUTED_SCALE = 2.5
NCORES = 8
NSLOT = 3                # routed expert slots per core
KT = D // 128            # 16 contraction tiles (gemm1)
IT = I // 128            # 11 contraction tiles (gemm2, routed)
DT = D // 128            # 16 output d-tiles (gemm2)
SSLICE = SI // NCORES    # 352 shared-intermediate rows per core
SIP = 384                # padded to 3x128
SIT = SIP // 128         # 3

f32 = mybir.dt.float32
bf16 = mybir.dt.bfloat16
nbf16 = ml_dtypes.bfloat16
ACT_SILU = mybir.ActivationFunctionType.Silu
ACT_COPY = mybir.ActivationFunctionType.Copy


# ---------------------------------------------------------------- routing
def _route(x, gate_w, bias):
    """Replicates the jax reference gate in numpy f32 (decision margins are
    >=1e-4 so ulp-level differences cannot flip the top-k).

    Returns topk_idx [T,6] int, weights [T,6] f32 (renormalized, unscaled).
    """
    logits = (x @ gate_w.T).astype(np.float32)
    scores = (1.0 / (1.0 + np.exp(-logits))).astype(np.float32)
    s_choice = scores + bias.astype(np.float32)
    grp = s_choice.reshape(T, N_GROUP, E // N_GROUP)
    group_scores = np.sort(grp, axis=2)[:, :, -2:].sum(2, dtype=np.float32)
    grp_idx = np.argsort(-group_scores, axis=1, kind="stable")[:, :TOPK_GROUP]
    gmask = np.zeros((T, N_GROUP), dtype=bool)
    gmask[np.arange(T)[:, None], grp_idx] = True
    emask = np.repeat(gmask, E // N_GROUP, axis=1)
    masked = np.where(emask, s_choice, -np.inf)
    topk_idx = np.argsort(-masked, axis=1, kind="stable")[:, :TOP_K]
    w = np.take_along_axis(scores, topk_idx, axis=1)
    w = (w / w.sum(axis=1, keepdims=True)).astype(np.float32)
    return topk_idx, w


# ---------------------------------------------------- slot capacity search
def _solve_caps(counts):
    """Pick slot capacities (a >= b >= c, multiples of 16, <= 512) with an
    exact DP feasibility check over the 8x3 cells; an expert may span
    several cells.  Returns (caps, per-expert cell usage [(na, nb, nc)])."""
    from functools import lru_cache
    counts = [int(k) for k in counts]
    ideal = sum(counts) / NCORES

    def solve(a, b, c):
        opts_per = []
        for k in counts:
            opts = []
            for na in range(0, 9):
                if na * a >= k + a and na > 0:
                    break
                for nb in range(0, 9):
                    if na * a + nb * b >= k + b and nb > 0:
                        break
                    for ncc in range(0, 9):
                        if na * a + nb * b + ncc * c >= k:
                            opts.append((na, nb, ncc))
                            break
            if not opts:
                return None
            opts_per.append(opts)

        @lru_cache(maxsize=None)
        def dp(i, ua, ub, uc):
            if i == len(counts):
                return ()
            for na, nb, ncc in opts_per[i]:
                if ua + na <= 8 and ub + nb <= 8 and uc + ncc <= 8:
                    r = dp(i + 1, ua + na, ub + nb, uc + ncc)
                    if r is not None:
                        return ((na, nb, ncc),) + r
            return None
        return dp(0, 0, 0, 0)

    best = None
    for a in range(256, 513, 8):
        for b in range(96, a + 1, 8):
            for c in range(16, b + 1, 8):
                s = a + b + c
                if s < ideal or (best is not None and s >= best[0][0]):
                    continue
                r = solve(a, b, c)
                if r is not None:
                    ncells = sum(sum(o) for o in r)
                    key = (s, ncells)
                    if best is None or key < best[0]:
                        best = (key, (a, b, c), r)
    if best is None:  # fallback: single big slot class per expert (always ok)
        a = min(512, 16 * ((max(counts) + 15) // 16))
        return (a, a, a), solve(a, a, a)
    return best[1], best[2]


# ------------------------------------------------------------ host packing
def _pack_wgu(w, it_cnt):
    """w: [2*ic, D] rows (gate block then up block, ic=128*it_cnt rows each)
    -> bf16 [2*it_cnt, 128, KT, 128] with gate/up 128-row tiles interleaved;
    tile m is w^T[k-tile, m-tile] with partition dim = contraction (d)."""
    ic = 128 * it_cnt
    g = w[:ic].reshape(it_cnt, 128, D)
    u = w[ic:].reshape(it_cnt, 128, D)
    inter = np.stack([g, u], axis=1).reshape(2 * it_cnt * 128, D)
    t = inter.T.reshape(KT, 128, 2 * it_cnt, 128).transpose(2, 1, 0, 3)
    return np.ascontiguousarray(t.astype(nbf16))


def _pack_wdT(wdT, it_cnt):
    """wdT: [128*it_cnt, D] (= w_down^T, zero-padded rows allowed)
    -> bf16 [DT, 128, it_cnt, 128]: stationary tiles [i-part, d] per
    (d-tile, i-k-tile) for the transposed GEMM2."""
    t = wdT.reshape(it_cnt, 128, DT, 128).transpose(2, 1, 0, 3)
    return np.ascontiguousarray(t.astype(nbf16))


def _pack_xT(xs, cap):
    """xs: [n, D] token rows -> bf16 [128, KT, cap] (x^T k-tiles, padded)."""
    out = np.zeros((128, KT, cap), dtype=nbf16)
    n = xs.shape[0]
    if n:
        out[:, :, :n] = xs.T.reshape(KT, 128, n).transpose(1, 0, 2).astype(nbf16)
    return out


# ------------------------------------------------------------ device build
def _build(caps):
    nc = bacc.Bacc("TRN2", target_bir_lowering=False, debug=False,
                   num_devices=NCORES)

    slot_in = []
    for s, c in enumerate(caps):
        slot_in.append(dict(
            xg=nc.dram_tensor(f"xg{s}", [128, KT, c], bf16, kind="ExternalInput"),
            wgu=nc.dram_tensor(f"wgu{s}", [2 * IT, 128, KT, 128], bf16, kind="ExternalInput"),
            wd=nc.dram_tensor(f"wd{s}", [DT, 128, IT, 128], bf16, kind="ExternalInput"),
            cwb=nc.dram_tensor(f"cwb{s}", [128, c], f32, kind="ExternalInput"),
            # output transposed + 4 d-tiles batched: [g][128][j][tok]
            yr=nc.dram_tensor(f"yr{s}", [DT // 4, 128, 4, c], bf16, kind="ExternalOutput"),
        ))
    xt_d = nc.dram_tensor("xt", [128, KT, T], bf16, kind="ExternalInput")
    wsgu_d = nc.dram_tensor("wsgu", [2 * SIT, 128, KT, 128], bf16, kind="ExternalInput")
    wsd_d = nc.dram_tensor("wsd", [128, DT, SIT, 128], bf16, kind="ExternalInput")
    ys_d = nc.dram_tensor("ys", [DT // 4, 2, 128, 4, 512], bf16, kind="ExternalOutput")

    with tile.TileContext(nc) as tc, ExitStack() as ctx:
        sb = ctx.enter_context(tc.tile_pool(name="sb", bufs=1))
        ps = ctx.enter_context(tc.tile_pool(name="ps", bufs=1, space="PSUM"))

        def load_x(xg_d, cap, sfx, eng=None, pieces=2):
            # x^T in `pieces` batched k-range loads so the first matmuls can
            # start while the rest streams in
            eng = eng or nc.gpsimd
            xg = sb.tile([128, KT, cap], bf16, tag=f"xg{sfx}", bufs=1, name=f"xg{sfx}")
            h = KT // pieces
            for q in range(pieces):
                eng.dma_start(xg[:, q * h:(q + 1) * h, :], xg_d.ap()[:, q * h:(q + 1) * h, :])
            return xg

        def gemm1(xat, cap, chunks, it_cnt, wgu_d, tag, hook=None):
            # GEMM1 + silu*mul -> at (A^T, [i, tokens], bf16)
            # xat(k, ci, off, n) yields the [128, n] moving x slice
            at = sb.tile([128, it_cnt, cap], bf16, tag=tag, bufs=1, name=tag)
            for t in range(it_cnt):
                pair = []
                for par in (0, 1):
                    wgu = sb.tile([128, KT, 128], bf16, tag="wgu", bufs=8, name="wgu")
                    nc.sync.dma_start(wgu[:], wgu_d.ap()[2 * t + par])
                    row = []
                    for ci, (off, n) in enumerate(chunks):
                        p = ps.tile([128, n], f32, tag=f"ps{par}", bufs=2, name=f"ps{par}")
                        for k in range(KT):
                            nc.tensor.matmul(p[:], wgu[:, k, :], xat(k, ci, off, n),
                                             start=(k == 0), stop=(k == KT - 1))
                        row.append(p)
                    pair.append(row)
                if hook is not None:
                    hook(t)
                for ci, (off, n) in enumerate(chunks):
                    tmp = sb.tile([128, n], f32, tag="tmp", bufs=2, name="tmp")
                    nc.scalar.activation(tmp[:], pair[0][ci][:], ACT_SILU)
                    nc.vector.tensor_mul(at[:, t, off:off + n], tmp[:], pair[1][ci][:])
            return at

        def preload_wd(s):
            # w_down tiles stream on the sync queue interleaved with wgu;
            # all 16 resident (45KB/partition) so no consumption gating
            wds = []
            def hook(t):
                while len(wds) < min(2 * (t + 1), DT):
                    wd = sb.tile([128, IT, 128], bf16, tag="wd", bufs=16, name="wd")
                    nc.sync.dma_start(wd[:], slot_in[s]["wd"].ap()[len(wds)])
                    wds.append(wd)
            return wds, hook

        def load_slot_x(s, cap, eng):
            io = slot_in[s]
            xg = load_x(io["xg"], cap, str(s), eng=eng)
            cwb = sb.tile([128, cap], f32, tag=f"cwb{s}", bufs=1, name=f"cwb{s}")
            eng.dma_start(cwb[:], io["cwb"].ap()[:])
            return xg, cwb

        def gemm2_routed(s, cap, at, cwb, wds):
            io = slot_in[s]
            # GEMM2 (transposed): out[d, tok] = w_down^T[i, d]^T @ at[i, tok]
            # 4 consecutive d-tiles batched into one output DMA
            for g in range(DT // 4):
                ysb4 = sb.tile([128, 4, cap], bf16, tag="ysb4", bufs=4, name="ysb4")
                for j in range(4):
                    dt = 4 * g + j
                    yp = ps.tile([128, cap], f32, tag="psy", bufs=4, name="yp")
                    for k in range(IT):
                        nc.tensor.matmul(yp[:], wds[dt][:, k, :], at[:, k, :],
                                         start=(k == 0), stop=(k == IT - 1))
                    nc.vector.tensor_mul(ysb4[:, j, :], yp[:], cwb[:])
                nc.scalar.dma_start(io["yr"].ap()[g], ysb4[:])

        def gemm2_shared(at, wsd):
            for ci, (off, n) in enumerate([(0, 512), (512, 512)]):
                for g in range(DT // 4):
                    ysb4 = sb.tile([128, 4, 512], bf16, tag="ysb4s", bufs=2, name="ysb4s")
                    for j in range(4):
                        dt = 4 * g + j
                        yp = ps.tile([128, 512], f32, tag="psy", bufs=4, name="yp")
                        for k in range(SIT):
                            nc.tensor.matmul(yp[:], wsd[:, dt, k, :], at[:, k, off:off + n],
                                             start=(k == 0), stop=(k == SIT - 1))
                        # drain alternates vector/scalar: a lone vector CAST
                        # (691ns) is slower than the 3-matmul group (640ns)
                        # and would throttle this phase
                        if j % 2:
                            nc.scalar.activation(ysb4[:, j, :], yp[:], ACT_COPY)
                        else:
                            nc.vector.tensor_copy(ysb4[:, j, :], yp[:])
                    # alternate the two HW queues so the tail drains in parallel
                    eng = nc.sync if ((4 * ci + g) % 2) else nc.scalar
                    eng.dma_start(ys_d.ap()[g, ci], ysb4[:])

        # Phase order: shared GEMM1 runs FIRST as a warm-up curtain — it
        # consumes weights at only ~75 GB/s for ~40us, giving the slot-0
        # weight streams a deep prefetch head start — and shared GEMM2 runs
        # LAST, a weight-free phase under which the output queues drain.
        # All x-side loads ride the scalar HW queue; their triggers sit
        # between the ACT instructions of the preceding GEMM1 phase, which
        # throttles each prefetch to fire one phase ahead of use.
        xt = load_x(xt_d, T, "xts", eng=nc.scalar, pieces=4)
        xg0, cwb0 = load_slot_x(0, caps[0], nc.scalar)
        at_s = gemm1(lambda k, ci, off, n: xt[:, k, off:off + n], T,
                     [(0, 512), (512, 512)], SIT, wsgu_d, "atS")
        def xacc(xg):
            return lambda k, ci, off, n: xg[:, k, off:off + n]

        xg1, cwb1 = load_slot_x(1, caps[1], nc.scalar)
        wds0, hook0 = preload_wd(0)
        at0 = gemm1(xacc(xg0), caps[0], [(0, caps[0])], IT, slot_in[0]["wgu"], "atA", hook0)
        gemm2_routed(0, caps[0], at0, cwb0, wds0)
        # shared w_down (all 16 tiles resident, 12KB/partition): emitted here
        # so its trigger fires ~125us in, after slot-0's outputs — NOT at t=0
        # where it would steal bandwidth from the critical xt/wsgu path
        wsd = sb.tile([128, DT, SIT, 128], bf16, tag="wsd", bufs=1, name="wsd")
        nc.scalar.dma_start(wsd[:], wsd_d.ap()[:])
        xg2, cwb2 = load_slot_x(2, caps[2], nc.scalar)
        wds1, hook1 = preload_wd(1)
        at1 = gemm1(xacc(xg1), caps[1], [(0, caps[1])], IT, slot_in[1]["wgu"], "atB", hook1)
        gemm2_routed(1, caps[1], at1, cwb1, wds1)
        wds2, hook2 = preload_wd(2)
        at2 = gemm1(xacc(xg2), caps[2], [(0, caps[2])], IT, slot_in[2]["wgu"], "atA", hook2)
        gemm2_routed(2, caps[2], at2, cwb2, wds2)
        gemm2_shared(at_s, wsd)

    nc.compile()
    return nc


# ----------------------------------------------------------------- kernel
def kernel(x, gate_w, bias, w_gate_up, w_down, shared_w_gate_up,
           shared_w_down, _trace=False):
    x = np.ascontiguousarray(x, dtype=np.float32)
    topk_idx, w = _route(x, gate_w, bias)
    cw_full = w.astype(np.float32) * np.float32(ROUTED_SCALE)

    # expert -> token list + weight list
    toks, wts, counts = [], [], np.zeros(E, dtype=np.int64)
    for e in range(E):
        tsel, ksel = np.where(topk_idx == e)
        toks.append(tsel)
        wts.append(cw_full[tsel, ksel])
        counts[e] = len(tsel)

    caps, usage = _solve_caps(counts)

    # split each expert's tokens into cell pieces; distribute cells to cores
    # cells[s] = list over cores of (expert_id, tok_idx, weights)
    cells = [[] for _ in range(NSLOT)]
    for e in range(E):
        na_nb_nc = usage[e]
        pos = 0
        for s in range(NSLOT):
            for _ in range(na_nb_nc[s]):
                take = min(caps[s], counts[e] - pos)
                if take < 0:
                    take = 0
                cells[s].append((e, toks[e][pos:pos + take], wts[e][pos:pos + take]))
                pos += take
    for s in range(NSLOT):
        assert len(cells[s]) <= NCORES, (caps, [len(c) for c in cells])
        while len(cells[s]) < NCORES:
            cells[s].append((0, np.zeros(0, np.int64), np.zeros(0, np.float32)))

    # pack each expert's weights once (bf16); cells alias these arrays
    packed_wgu = {}
    packed_wd = {}
    for e in set(c[0] for s in range(NSLOT) for c in cells[s]):
        packed_wgu[e] = _pack_wgu(w_gate_up[e], IT)
        packed_wd[e] = _pack_wdT(np.ascontiguousarray(w_down[e].T), IT)

    xt_packed = _pack_xT(x, T)
    in_maps = []
    for core in range(NCORES):
        m = {}
        for s, c in enumerate(caps):
            eid, tsel, tw = cells[s][core]
            m[f"xg{s}"] = _pack_xT(x[tsel], c)
            m[f"wgu{s}"] = packed_wgu[eid]
            m[f"wd{s}"] = packed_wd[eid]
            cwb = np.zeros((128, c), dtype=np.float32)
            cwb[:, :len(tw)] = tw[None, :]
            m[f"cwb{s}"] = cwb
        # shared expert slice (rows [352c, 352c+352), zero-padded to 384)
        gsl = np.zeros((2 * SIP, D), dtype=np.float32)
        gsl[:SSLICE] = shared_w_gate_up[SSLICE * core: SSLICE * (core + 1)]
        gsl[SIP:SIP + SSLICE] = shared_w_gate_up[SI + SSLICE * core: SI + SSLICE * (core + 1)]
        m["wsgu"] = _pack_wgu(gsl, SIT)
        sdT = np.zeros((SIP, D), dtype=np.float32)
        sdT[:SSLICE] = shared_w_down[:, SSLICE * core: SSLICE * (core + 1)].T
        m["wsd"] = np.ascontiguousarray(_pack_wdT(sdT, SIT).transpose(1, 0, 2, 3))
        m["xt"] = xt_packed
        in_maps.append(m)

    nc = _build(caps)
    kw = {}
    if _trace:
        kw = dict(trace=True, trace_cores=list(range(NCORES)))
    res = run_bass_kernel_spmd(nc, in_maps, core_ids=list(range(NCORES)), **kw)

    y = np.zeros((T, D), dtype=np.float32)
    for core in range(NCORES):
        # ys: [g, ci, p, j, u] -> [d = g*512 + j*128 + p, t = ci*512 + u]
        ys = res.results[core]["ys"].astype(np.float32)
        y += ys.transpose(0, 3, 2, 1, 4).reshape(D, T).T
    for core in range(NCORES):
        for s in range(NSLOT):
            eid, tsel, _ = cells[s][core]
            n = len(tsel)
            if n:
                # yr: [g, p, j, tok] -> [d = g*512 + j*128 + p, tok]
                yr = res.results[core][f"yr{s}"].astype(np.float32)
                c = yr.shape[-1]
                y[tsel] += yr.transpose(0, 2, 1, 3).reshape(D, c)[:, :n].T
    if _trace:
        return y, res
    return y
